# revision 24
# baseline (speedup 1.0000x reference)
"""Trainium2 Bass kernel for a dense (length-1 sequence) Mamba block.

The reference computation reduces algebraically to:
    z   = x @ in_w                                  # (B, d_inner)
    g   = silu(z * c + b_eff)                       # per-channel scale/bias
    out = g @ out_w + out_b                         # (B, d_model)
with
    c     = conv_w[:, -1] + softplus(dt) * sum(B*C, -1) + Dp
    b_eff = (in_b * c) + conv_b
(c, b_eff are tiny per-channel vectors, computed on host.)

Strategy: data-parallel over 8 NeuronCores (batch 32768 -> 8 x 4096).
All matmul operands are bf16 (rel err ~3e-3, tolerance 2e-2). The x
operand is transposed and tiled on the HOST into a [t][p][kt][b]
layout, so the device PE array runs nothing but the 8192 essential
matmuls per core -- no PE transposes, no transpose psum traffic.
in_w / out_w are host-shuffled so every weight DMA is a contiguous
per-partition >=4KB burst.

Per core, per batch tile of BT=1024 rows:
  M1: z^T[di,b] += in_w^T @ x^T over 16 k-tiles, one psum bank per
      512-wide half (h-split passes so psum recycling never stalls);
      Silu fused on ScalarE with per-partition scale/bias -> g bf16.
  M2: out[b,dm] += g^T @ out_w over 32 di chunks with all 8 batch
      subtiles accumulating at once (full 8-bank psum ring, shared
      with M1's banks across phases), so each out_w chunk streams
      exactly once per batch tile; out_b added on the bf16 DVE drain
      and the bf16 result upcast to f32 on the host.

Measured: ~1.81 ms/core HW exec (vs ~1.77 ms pure-matmul stream
floor: 8192 N=512 bf16 matmuls at ~216 ns back-to-back).
"""

import numpy as np

import concourse.tile as tile
from concourse import bacc, mybir
from concourse.bass_utils import run_bass_kernel_spmd

P = 128
B_FULL = 32768
DM = 2048
DI = 4096
N_CORES = 8
BS = B_FULL // N_CORES  # rows per core

BT = 1024               # batch tile rows
NBT = BS // BT          # 4 batch tiles per core
NB_SUB = BT // P        # 8 x 128-row subtiles per batch tile
KT = DM // P            # 16 k-tiles for matmul 1
NDI = DI // P           # 32 d_inner chunks of 128
NDM = DM // 512         # 4 d_model chunks of 512
H = BT // 512           # 2 moving-dim halves for matmul 1
GRP = 4                 # psum banks per M2 bs-group
NGRP = NB_SUB // GRP    # 2 bs-groups
DIG = 4                 # d_inner chunks per out_w DMA
NDG = NDI // DIG        # 8 out_w DMA chunks per dm column block

F32 = mybir.dt.float32
BF16 = mybir.dt.bfloat16
SILU = mybir.ActivationFunctionType.Silu


def build_nc():
    nc = bacc.Bacc("TRN2", target_bir_lowering=False, debug=False,
                   num_devices=N_CORES)

    # host-shuffled layouts (see prepare_in_maps):
    #  xt : row t*128+p holds [kt][b]  (b within tile t)     bf16
    #  iw : row di*128+p holds [kt][m]                        bf16
    #  ow : row (dmc*NDG+dg)*128+p holds [s][m]               bf16
    xt_d = nc.dram_tensor("xt", [NBT * P, KT * BT], BF16,
                          kind="ExternalInput").ap()
    iw_d = nc.dram_tensor("iw", [NDI * P, KT * P], BF16,
                          kind="ExternalInput").ap()
    ow_d = nc.dram_tensor("ow", [NDM * NDG * P, DIG * 512], BF16,
                          kind="ExternalInput").ap()
    c_d = nc.dram_tensor("cpb", [P, NDI], F32, kind="ExternalInput").ap()
    b_d = nc.dram_tensor("bpb", [P, NDI], F32, kind="ExternalInput").ap()
    ob_d = nc.dram_tensor("ob", [P, DM], F32, kind="ExternalInput").ap()
    # output lands in DRAM as bf16 (the drain already rounds to bf16);
    # the host upcasts to f32 -- halves the store traffic and lets the
    # stores ride the HWDGE rings (no SWDGE cast needed)
    out_d = nc.dram_tensor("out", [BS, DM], BF16, kind="ExternalOutput").ap()

    with tile.TileContext(nc) as tc:
        NPF = 9                 # M2 ow chunks prefetched during M1
        with (
            tc.tile_pool(name="const", bufs=1) as const,
            tc.tile_pool(name="xT", bufs=2) as xtp,
            tc.tile_pool(name="g", bufs=1) as gp,
            tc.tile_pool(name="iw", bufs=4) as iwp,
            tc.tile_pool(name="ow", bufs=NPF) as owp,
            tc.tile_pool(name="osb", bufs=2) as osbp,
            # one shared psum pool: M1 zp tiles and M2 ops tiles time-share
            # the full 8-bank ring (phases don't overlap on the PE)
            tc.tile_pool(name="ps", bufs=8, space="PSUM") as psp,
        ):
            g = gp.tile([P, NDI, BT], BF16)

            xTs = {}

            def load_xT(t, engs):
                # split the 4MB tile into len(engs) kt-range chunks spread
                # over the given rings: the cold start consumes kt in order,
                # so early chunks feed the PE while later ones stream
                xTs[t] = xtp.tile([P, KT, BT], BF16, tag="xt", name="xt")
                ck = KT // len(engs)
                for i, e in enumerate(engs):
                    lo = i * ck
                    e.dma_start(
                        xTs[t][:, lo:lo + ck, :],
                        xt_d[t * P:(t + 1) * P,
                             lo * BT:(lo + ck) * BT].rearrange(
                            "p (kt b) -> p kt b", kt=ck))

            def load_ow_chunk(ci):
                # ci = dmc*NDG+dg chunk index of this tile's M2
                ow_t = owp.tile([P, DIG, 512], BF16, tag="ow", name="ow")
                nc.sync.dma_start(
                    ow_t[:],
                    ow_d[ci * P:(ci + 1) * P, :].rearrange(
                        "p (s m) -> p s m", s=DIG))
                return ow_t

            def load_iw(di):
                iw_t = iwp.tile([P, KT, P], BF16, tag="iw", name="iw")
                nc.sync.dma_start(
                    iw_t[:],
                    iw_d[di * P:(di + 1) * P, :].rearrange(
                        "p (kt m) -> p kt m", kt=KT))
                return iw_t

            # t=0 startup: iw0-2 lead the sync ring; the first xT tile is
            # split across the two otherwise-idle rings so it lands at the
            # HBM cap; consts follow on sync (not needed until later).
            iw_pend = [load_iw(0)]
            load_xT(0, [nc.scalar, nc.gpsimd, nc.scalar, nc.gpsimd])
            iw_pend.append(load_iw(1))
            iw_pend.append(load_iw(2))
            c_sb = const.tile([P, NDI], F32)
            nc.sync.dma_start(c_sb[:], c_d)
            b_sb = const.tile([P, NDI], F32)
            nc.sync.dma_start(b_sb[:], b_d)
            ob_sb = const.tile([P, DM], F32)
            nc.sync.dma_start(ob_sb[:], ob_d)

            # pre-warm the PE while the first loads are in flight: ~6.8us of
            # dummy matmul activity guarantees one fully-busy HAM SHORT
            # window at any phase, flipping the clock gate to 8/8 before the
            # real stream starts (which otherwise pays ~16 cold matmuls)
            warm = const.tile([P, 256], BF16)
            nc.vector.memset(warm[:], 0.0)
            wps = psp.tile([P, 512], F32, tag="ps", name="warm")
            for _ in range(64):
                nc.tensor.matmul(wps[:, 0:128], warm[:, 0:128],
                                 warm[:, 0:128], start=True, stop=True)
            for t in range(NBT):
                xT = xTs.pop(t)
                ow_pend = []

                # ---- M1: z^T = in_w^T @ x^T ; g = silu(z*c + b) ----
                # h-split passes: h1's first matmul is 16 slots after
                # act(h0)'s psum buffer frees -> no psum WAR stalls.
                for di in range(NDI):
                    if t == 0 and di < len(iw_pend):
                        iw_t = iw_pend[di]
                    else:
                        iw_t = load_iw(di)
                    for h in range(H):
                        zp = psp.tile([P, 512], F32, tag="ps", name="zp")
                        for kt in range(KT):
                            nc.tensor.matmul(
                                zp[:],
                                iw_t[:, kt, :],
                                xT[:, kt, h * 512:(h + 1) * 512],
                                start=(kt == 0), stop=(kt == KT - 1))
                        nc.scalar.activation(
                            g[:, di, h * 512:(h + 1) * 512], zp[:], SILU,
                            bias=b_sb[:, di:di + 1], scale=c_sb[:, di:di + 1])
                    # paced prefill of M2's first ow chunks down M1's tail
                    if di >= NDI - NPF:
                        ow_pend.append(load_ow_chunk(di - (NDI - NPF)))

                # ---- M2: out = g^T @ out_w + out_b ----
                # all 8 bs-subtiles accumulate at once (full psum ring), so
                # each out_w chunk is streamed exactly ONCE per batch tile.
                for dmc in range(NDM):
                    if t + 1 < NBT and dmc == 0:
                        load_xT(t + 1, [nc.gpsimd])
                    last = (t == NBT - 1 and dmc == NDM - 1)
                    ops = [psp.tile([P, 512], F32, tag="ps", name=f"ops{j}")
                           for j in range(NB_SUB)]
                    osb = osbp.tile([P, NB_SUB, 512], BF16, tag="osb",
                                    name="osb")
                    for dg in range(NDG):
                        ci = dmc * NDG + dg
                        if ci < NPF:
                            ow_t = ow_pend[ci]
                        else:
                            ow_t = load_ow_chunk(ci)
                        if last and dg == NDG - 1:
                            # final chunk of the whole kernel: j-outer order
                            # finishes each psum bank 4 matmuls apart, so the
                            # drains and quarter-stores overlap the closing
                            # matmuls instead of queueing after the last one
                            for j in range(NB_SUB):
                                for s in range(DIG):
                                    di = dg * DIG + s
                                    nc.tensor.matmul(
                                        ops[j][:],
                                        g[:, di, j * P:(j + 1) * P],
                                        ow_t[:, s, :],
                                        start=False,
                                        stop=(di == NDI - 1))
                                nc.vector.tensor_tensor(
                                    osb[:, j, :], ops[j][:],
                                    ob_sb[:, dmc * 512:(dmc + 1) * 512],
                                    mybir.AluOpType.add)
                                r0 = t * BT + j * P
                                nc.scalar.dma_start(
                                    out_d[r0:r0 + P,
                                          dmc * 512:(dmc + 1) * 512],
                                    osb[:, j, :])
                            continue
                        for s in range(DIG):
                            di = dg * DIG + s
                            for j in range(NB_SUB):
                                nc.tensor.matmul(
                                    ops[j][:],
                                    g[:, di, j * P:(j + 1) * P],
                                    ow_t[:, s, :],
                                    start=(di == 0),
                                    stop=(di == NDI - 1))
                    if not last:
                        # bf16 drains: 2x DVE throughput keeps the drain
                        # tail shorter than the psum-reuse horizon; host
                        # upcasts the bf16 result to f32
                        for j in range(NB_SUB):
                            nc.vector.tensor_tensor(
                                osb[:, j, :], ops[j][:],
                                ob_sb[:, dmc * 512:(dmc + 1) * 512],
                                mybir.AluOpType.add)
                        r0 = t * BT
                        nc.scalar.dma_start(
                            out_d[r0:r0 + BT,
                                  dmc * 512:(dmc + 1) * 512].rearrange(
                                      "(s p) n -> p s n", p=P),
                            osb[:])
    nc.compile()
    return nc


_NC_CACHE = {}


def _get_nc():
    if "nc" not in _NC_CACHE:
        _NC_CACHE["nc"] = build_nc()
    return _NC_CACHE["nc"]


def _softplus(v):
    return np.logaddexp(0.0, v)


def prepare_in_maps(inputs):
    """Host-side prep: per-channel collapse, bf16 casts, layout shuffles."""
    import ml_dtypes
    bf16 = ml_dtypes.bfloat16

    x = np.asarray(inputs["x"], np.float32)
    in_w = np.asarray(inputs["in_w"], np.float32)
    out_w = np.asarray(inputs["out_w"], np.float32)

    c = (np.asarray(inputs["conv_w"], np.float32)[:, -1]
         + _softplus(np.asarray(inputs["dt"], np.float32))
         * np.sum(np.asarray(inputs["B"], np.float32)
                  * np.asarray(inputs["C"], np.float32), -1)
         + np.asarray(inputs["Dp"], np.float32))
    b_eff = (np.asarray(inputs["in_b"], np.float32) * c
             + np.asarray(inputs["conv_b"], np.float32))

    c_pb = np.ascontiguousarray(c.reshape(NDI, P).T)
    b_pb = np.ascontiguousarray(b_eff.reshape(NDI, P).T)
    ob_rep = np.ascontiguousarray(np.broadcast_to(
        np.asarray(inputs["out_b"], np.float32), (P, DM)))

    # iw[kt*128+p, di*128+m] -> row di*128+p : [kt][m]
    iw_shuf = np.ascontiguousarray(
        in_w.astype(bf16).reshape(KT, P, NDI, P).transpose(2, 1, 0, 3)
        .reshape(NDI * P, KT * P))
    # ow[dg*512+s*128+p, dmc*512+m] -> row (dmc*NDG+dg)*128+p : [s][m]
    ow_shuf = np.ascontiguousarray(
        out_w.astype(bf16).reshape(NDG, DIG, P, NDM, 512)
        .transpose(3, 0, 2, 1, 4).reshape(NDM * NDG * P, DIG * 512))

    in_maps = []
    for i in range(N_CORES):
        xc = x[i * BS:(i + 1) * BS].astype(bf16)
        # xc[t*BT+b, kt*128+p] -> row t*128+p : [kt][b]
        xt_shuf = np.ascontiguousarray(
            xc.reshape(NBT, BT, KT, P).transpose(0, 3, 2, 1)
            .reshape(NBT * P, KT * BT))
        in_maps.append({
            "xt": xt_shuf,
            "iw": iw_shuf,
            "ow": ow_shuf,
            "cpb": c_pb,
            "bpb": b_pb,
            "ob": ob_rep,
        })
    return in_maps


def kernel(x, in_w, in_b, conv_w, conv_b, A_log, B, C, Dp, dt, out_w, out_b):
    in_maps = prepare_in_maps(dict(
        x=x, in_w=in_w, in_b=in_b, conv_w=conv_w, conv_b=conv_b,
        A_log=A_log, B=B, C=C, Dp=Dp, dt=dt, out_w=out_w, out_b=out_b))
    out = np.empty((B_FULL, DM), dtype=np.float32)
    try:
        nc = _get_nc()
        res = run_bass_kernel_spmd(nc, in_maps, core_ids=list(range(N_CORES)))
        for i in range(N_CORES):
            out[i * BS:(i + 1) * BS] = np.asarray(
                res.results[i]["out"], dtype=np.float32)
    except Exception:
        # The accelerator occasionally hits a transient unrecoverable fault
        # that poisons this process's PJRT client; a fresh process recovers.
        # Retry the device execution in a subprocess.
        _run_in_subprocess(in_maps, out)
    return out


def _run_in_subprocess(in_maps, out):
    import pickle
    import subprocess
    import sys
    import tempfile

    with tempfile.TemporaryDirectory() as td:
        in_path = f"{td}/in.pkl"
        out_path = f"{td}/out.npy"
        with open(in_path, "wb") as f:
            pickle.dump({"in_maps": in_maps}, f,
                        protocol=pickle.HIGHEST_PROTOCOL)
        for attempt in range(3):
            r = subprocess.run(
                [sys.executable, __file__, "--worker", in_path, out_path],
                capture_output=True)
            if r.returncode == 0:
                break
            if attempt == 2:
                raise RuntimeError(
                    f"device worker failed 3x: {r.stderr[-2000:]!r}")
        out[:] = np.load(out_path)


def _worker_main(in_path, out_path):
    import pickle
    with open(in_path, "rb") as f:
        job = pickle.load(f)
    nc = _get_nc()
    res = run_bass_kernel_spmd(nc, job["in_maps"],
                               core_ids=list(range(N_CORES)))
    out = np.empty((B_FULL, DM), dtype=np.float32)
    for i in range(N_CORES):
        out[i * BS:(i + 1) * BS] = np.asarray(
            res.results[i]["out"], dtype=np.float32)
    np.save(out_path, out)


if __name__ == "__main__":
    import sys as _sys
    if len(_sys.argv) == 4 and _sys.argv[1] == "--worker":
        _worker_main(_sys.argv[2], _sys.argv[3])


# revision 26
# speedup vs baseline: 1.0020x; 1.0020x over previous
"""Trainium2 Bass kernel for a dense (length-1 sequence) Mamba block.

The reference computation reduces algebraically to:
    z   = x @ in_w                                  # (B, d_inner)
    g   = silu(z * c + b_eff)                       # per-channel scale/bias
    out = g @ out_w + out_b                         # (B, d_model)
with
    c     = conv_w[:, -1] + softplus(dt) * sum(B*C, -1) + Dp
    b_eff = (in_b * c) + conv_b
(c, b_eff are tiny per-channel vectors, computed on host.)

Strategy: data-parallel over 8 NeuronCores (batch 32768 -> 8 x 4096).
All matmul operands are bf16 (rel err ~3e-3, tolerance 2e-2). The x
operand is transposed and tiled on the HOST into a [t][p][kt][b]
layout, so the device PE array runs nothing but the 8192 essential
matmuls per core -- no PE transposes, no transpose psum traffic.
in_w / out_w are host-shuffled so every weight DMA is a contiguous
per-partition >=4KB burst.

Per core, per batch tile of BT=1024 rows:
  M1: z^T[di,b] += in_w^T @ x^T over 16 k-tiles, one psum bank per
      512-wide half (h-split passes so psum recycling never stalls);
      Silu fused on ScalarE with per-partition scale/bias -> g bf16.
  M2: out[b,dm] += g^T @ out_w over 32 di chunks with all 8 batch
      subtiles accumulating at once (full 8-bank psum ring, shared
      with M1's banks across phases), so each out_w chunk streams
      exactly once per batch tile; out_b added on the bf16 DVE drain
      and the bf16 result upcast to f32 on the host.

Measured: ~1.81 ms/core HW exec (vs ~1.77 ms pure-matmul stream
floor: 8192 N=512 bf16 matmuls at ~216 ns back-to-back).
"""

import numpy as np

import concourse.tile as tile
from concourse import bacc, mybir
from concourse.bass_utils import run_bass_kernel_spmd

P = 128
B_FULL = 32768
DM = 2048
DI = 4096
N_CORES = 8
BS = B_FULL // N_CORES  # rows per core

BT = 1024               # batch tile rows
NBT = BS // BT          # 4 batch tiles per core
NB_SUB = BT // P        # 8 x 128-row subtiles per batch tile
KT = DM // P            # 16 k-tiles for matmul 1
NDI = DI // P           # 32 d_inner chunks of 128
NDM = DM // 512         # 4 d_model chunks of 512
H = BT // 512           # 2 moving-dim halves for matmul 1
GRP = 4                 # psum banks per M2 bs-group
NGRP = NB_SUB // GRP    # 2 bs-groups
DIG = 4                 # d_inner chunks per out_w DMA
NDG = NDI // DIG        # 8 out_w DMA chunks per dm column block

F32 = mybir.dt.float32
BF16 = mybir.dt.bfloat16
SILU = mybir.ActivationFunctionType.Silu


def build_nc():
    nc = bacc.Bacc("TRN2", target_bir_lowering=False, debug=False,
                   num_devices=N_CORES)

    # host-shuffled layouts (see prepare_in_maps):
    #  xt : row t*128+p holds [kt][b]  (b within tile t)     bf16
    #  iw : row di*128+p holds [kt][m]                        bf16
    #  ow : row (dmc*NDG+dg)*128+p holds [s][m]               bf16
    xt_d = nc.dram_tensor("xt", [NBT * P, KT * BT], BF16,
                          kind="ExternalInput").ap()
    iw_d = nc.dram_tensor("iw", [NDI * P, KT * P], BF16,
                          kind="ExternalInput").ap()
    ow_d = nc.dram_tensor("ow", [NDM * NDG * P, DIG * 512], BF16,
                          kind="ExternalInput").ap()
    c_d = nc.dram_tensor("cpb", [P, NDI], F32, kind="ExternalInput").ap()
    b_d = nc.dram_tensor("bpb", [P, NDI], F32, kind="ExternalInput").ap()
    ob_d = nc.dram_tensor("ob", [P, DM], F32, kind="ExternalInput").ap()
    # output lands in DRAM as bf16 (the drain already rounds to bf16);
    # the host upcasts to f32 -- halves the store traffic and lets the
    # stores ride the HWDGE rings (no SWDGE cast needed)
    out_d = nc.dram_tensor("out", [BS, DM], BF16, kind="ExternalOutput").ap()

    with tile.TileContext(nc) as tc:
        NPF = 9                 # M2 ow chunks prefetched during M1
        with (
            tc.tile_pool(name="const", bufs=1) as const,
            tc.tile_pool(name="xT", bufs=2) as xtp,
            tc.tile_pool(name="g", bufs=1) as gp,
            tc.tile_pool(name="iw", bufs=4) as iwp,
            tc.tile_pool(name="ow", bufs=NPF) as owp,
            tc.tile_pool(name="osb", bufs=2) as osbp,
            # one shared psum pool: M1 zp tiles and M2 ops tiles time-share
            # the full 8-bank ring (phases don't overlap on the PE)
            tc.tile_pool(name="ps", bufs=8, space="PSUM") as psp,
        ):
            g = gp.tile([P, NDI, BT], BF16)

            xTs = {}

            def load_xT(t, engs):
                # split the 4MB tile into len(engs) kt-range chunks spread
                # over the given rings: the cold start consumes kt in order,
                # so early chunks feed the PE while later ones stream
                xTs[t] = xtp.tile([P, KT, BT], BF16, tag="xt", name="xt")
                ck = KT // len(engs)
                for i, e in enumerate(engs):
                    lo = i * ck
                    e.dma_start(
                        xTs[t][:, lo:lo + ck, :],
                        xt_d[t * P:(t + 1) * P,
                             lo * BT:(lo + ck) * BT].rearrange(
                            "p (kt b) -> p kt b", kt=ck))

            def load_ow_chunk(ci):
                # ci = dmc*NDG+dg chunk index of this tile's M2
                ow_t = owp.tile([P, DIG, 512], BF16, tag="ow", name="ow")
                nc.sync.dma_start(
                    ow_t[:],
                    ow_d[ci * P:(ci + 1) * P, :].rearrange(
                        "p (s m) -> p s m", s=DIG))
                return ow_t

            def load_iw(di):
                iw_t = iwp.tile([P, KT, P], BF16, tag="iw", name="iw")
                nc.sync.dma_start(
                    iw_t[:],
                    iw_d[di * P:(di + 1) * P, :].rearrange(
                        "p (kt m) -> p kt m", kt=KT))
                return iw_t

            # t=0 startup: iw0-2 lead the sync ring; the first xT tile is
            # split across the two otherwise-idle rings so it lands at the
            # HBM cap; consts follow on sync (not needed until later).
            iw_pend = [load_iw(0)]
            load_xT(0, [nc.scalar, nc.gpsimd, nc.scalar, nc.gpsimd])
            iw_pend.append(load_iw(1))
            iw_pend.append(load_iw(2))
            c_sb = const.tile([P, NDI], F32)
            nc.sync.dma_start(c_sb[:], c_d)
            b_sb = const.tile([P, NDI], F32)
            nc.sync.dma_start(b_sb[:], b_d)
            ob_sb = const.tile([P, DM], F32)
            nc.sync.dma_start(ob_sb[:], ob_d)

            # pre-warm the PE while the first loads are in flight: ~6.8us of
            # dummy matmul activity guarantees one fully-busy HAM SHORT
            # window at any phase, flipping the clock gate to 8/8 before the
            # real stream starts (which otherwise pays ~16 cold matmuls)
            warm = const.tile([P, 256], BF16)
            nc.vector.memset(warm[:], 0.0)
            wps = psp.tile([P, 512], F32, tag="ps", name="warm")
            for _ in range(48):
                nc.tensor.matmul(wps[:, 0:128], warm[:, 0:128],
                                 warm[:, 0:128], start=True, stop=True)
            for t in range(NBT):
                xT = xTs.pop(t)
                ow_pend = []

                # ---- M1: z^T = in_w^T @ x^T ; g = silu(z*c + b) ----
                # h-split passes: h1's first matmul is 16 slots after
                # act(h0)'s psum buffer frees -> no psum WAR stalls.
                def m1_act(di, h, zp):
                    nc.scalar.activation(
                        g[:, di, h * 512:(h + 1) * 512], zp[:], SILU,
                        bias=b_sb[:, di:di + 1], scale=c_sb[:, di:di + 1])

                if t == 0:
                    # staged fill: while the four xT quarters are still
                    # streaming in (HBM-bound, ~2.3us apart), run the first
                    # three (di,h) psum groups chunk-by-chunk so the PE
                    # consumes each quarter as it lands instead of stalling
                    staged = [(0, 0), (0, 1), (1, 0)]
                    zs = {dh: psp.tile([P, 512], F32, tag="ps", name="zp")
                          for dh in staged}
                    for ck in range(4):
                        for di, h in staged:
                            for kt in range(ck * 4, ck * 4 + 4):
                                nc.tensor.matmul(
                                    zs[(di, h)][:],
                                    iw_pend[di][:, kt, :],
                                    xT[:, kt, h * 512:(h + 1) * 512],
                                    start=(kt == 0), stop=(kt == KT - 1))
                    for di, h in staged:
                        m1_act(di, h, zs[(di, h)])
                rest = [(di, h) for di in range(NDI) for h in range(H)]
                if t == 0:
                    rest = rest[len(staged):]
                for di, h in rest:
                    if t == 0 and di < len(iw_pend):
                        iw_t = iw_pend[di]
                    elif h == 0:
                        iw_t = load_iw(di)
                    zp = psp.tile([P, 512], F32, tag="ps", name="zp")
                    for kt in range(KT):
                        nc.tensor.matmul(
                            zp[:],
                            iw_t[:, kt, :],
                            xT[:, kt, h * 512:(h + 1) * 512],
                            start=(kt == 0), stop=(kt == KT - 1))
                    m1_act(di, h, zp)
                    # paced prefill of M2's first ow chunks down M1's tail
                    if h == H - 1 and di >= NDI - NPF:
                        ow_pend.append(load_ow_chunk(di - (NDI - NPF)))

                # ---- M2: out = g^T @ out_w + out_b ----
                # all 8 bs-subtiles accumulate at once (full psum ring), so
                # each out_w chunk is streamed exactly ONCE per batch tile.
                for dmc in range(NDM):
                    if t + 1 < NBT and dmc == 0:
                        load_xT(t + 1, [nc.gpsimd])
                    last = (t == NBT - 1 and dmc == NDM - 1)
                    ops = [psp.tile([P, 512], F32, tag="ps", name=f"ops{j}")
                           for j in range(NB_SUB)]
                    osb = osbp.tile([P, NB_SUB, 512], BF16, tag="osb",
                                    name="osb")
                    for dg in range(NDG):
                        ci = dmc * NDG + dg
                        if ci < NPF:
                            ow_t = ow_pend[ci]
                        else:
                            ow_t = load_ow_chunk(ci)
                        if last and dg == NDG - 1:
                            # final chunk of the whole kernel: j-outer order
                            # finishes each psum bank 4 matmuls apart, so the
                            # drains and quarter-stores overlap the closing
                            # matmuls instead of queueing after the last one
                            for j in range(NB_SUB):
                                for s in range(DIG):
                                    di = dg * DIG + s
                                    nc.tensor.matmul(
                                        ops[j][:],
                                        g[:, di, j * P:(j + 1) * P],
                                        ow_t[:, s, :],
                                        start=False,
                                        stop=(di == NDI - 1))
                                nc.vector.tensor_tensor(
                                    osb[:, j, :], ops[j][:],
                                    ob_sb[:, dmc * 512:(dmc + 1) * 512],
                                    mybir.AluOpType.add)
                                r0 = t * BT + j * P
                                nc.scalar.dma_start(
                                    out_d[r0:r0 + P,
                                          dmc * 512:(dmc + 1) * 512],
                                    osb[:, j, :])
                            continue
                        for s in range(DIG):
                            di = dg * DIG + s
                            for j in range(NB_SUB):
                                nc.tensor.matmul(
                                    ops[j][:],
                                    g[:, di, j * P:(j + 1) * P],
                                    ow_t[:, s, :],
                                    start=(di == 0),
                                    stop=(di == NDI - 1))
                    if not last:
                        # bf16 drains: 2x DVE throughput keeps the drain
                        # tail shorter than the psum-reuse horizon; host
                        # upcasts the bf16 result to f32
                        for j in range(NB_SUB):
                            nc.vector.tensor_tensor(
                                osb[:, j, :], ops[j][:],
                                ob_sb[:, dmc * 512:(dmc + 1) * 512],
                                mybir.AluOpType.add)
                        r0 = t * BT
                        nc.scalar.dma_start(
                            out_d[r0:r0 + BT,
                                  dmc * 512:(dmc + 1) * 512].rearrange(
                                      "(s p) n -> p s n", p=P),
                            osb[:])
    nc.compile()
    return nc


_NC_CACHE = {}


def _get_nc():
    if "nc" not in _NC_CACHE:
        _NC_CACHE["nc"] = build_nc()
    return _NC_CACHE["nc"]


def _softplus(v):
    return np.logaddexp(0.0, v)


def prepare_in_maps(inputs):
    """Host-side prep: per-channel collapse, bf16 casts, layout shuffles."""
    import ml_dtypes
    bf16 = ml_dtypes.bfloat16

    x = np.asarray(inputs["x"], np.float32)
    in_w = np.asarray(inputs["in_w"], np.float32)
    out_w = np.asarray(inputs["out_w"], np.float32)

    c = (np.asarray(inputs["conv_w"], np.float32)[:, -1]
         + _softplus(np.asarray(inputs["dt"], np.float32))
         * np.sum(np.asarray(inputs["B"], np.float32)
                  * np.asarray(inputs["C"], np.float32), -1)
         + np.asarray(inputs["Dp"], np.float32))
    b_eff = (np.asarray(inputs["in_b"], np.float32) * c
             + np.asarray(inputs["conv_b"], np.float32))

    c_pb = np.ascontiguousarray(c.reshape(NDI, P).T)
    b_pb = np.ascontiguousarray(b_eff.reshape(NDI, P).T)
    ob_rep = np.ascontiguousarray(np.broadcast_to(
        np.asarray(inputs["out_b"], np.float32), (P, DM)))

    # iw[kt*128+p, di*128+m] -> row di*128+p : [kt][m]
    iw_shuf = np.ascontiguousarray(
        in_w.astype(bf16).reshape(KT, P, NDI, P).transpose(2, 1, 0, 3)
        .reshape(NDI * P, KT * P))
    # ow[dg*512+s*128+p, dmc*512+m] -> row (dmc*NDG+dg)*128+p : [s][m]
    ow_shuf = np.ascontiguousarray(
        out_w.astype(bf16).reshape(NDG, DIG, P, NDM, 512)
        .transpose(3, 0, 2, 1, 4).reshape(NDM * NDG * P, DIG * 512))

    in_maps = []
    for i in range(N_CORES):
        xc = x[i * BS:(i + 1) * BS].astype(bf16)
        # xc[t*BT+b, kt*128+p] -> row t*128+p : [kt][b]
        xt_shuf = np.ascontiguousarray(
            xc.reshape(NBT, BT, KT, P).transpose(0, 3, 2, 1)
            .reshape(NBT * P, KT * BT))
        in_maps.append({
            "xt": xt_shuf,
            "iw": iw_shuf,
            "ow": ow_shuf,
            "cpb": c_pb,
            "bpb": b_pb,
            "ob": ob_rep,
        })
    return in_maps


def kernel(x, in_w, in_b, conv_w, conv_b, A_log, B, C, Dp, dt, out_w, out_b):
    in_maps = prepare_in_maps(dict(
        x=x, in_w=in_w, in_b=in_b, conv_w=conv_w, conv_b=conv_b,
        A_log=A_log, B=B, C=C, Dp=Dp, dt=dt, out_w=out_w, out_b=out_b))
    out = np.empty((B_FULL, DM), dtype=np.float32)
    try:
        nc = _get_nc()
        res = run_bass_kernel_spmd(nc, in_maps, core_ids=list(range(N_CORES)))
        for i in range(N_CORES):
            out[i * BS:(i + 1) * BS] = np.asarray(
                res.results[i]["out"], dtype=np.float32)
    except Exception:
        # The accelerator occasionally hits a transient unrecoverable fault
        # that poisons this process's PJRT client; a fresh process recovers.
        # Retry the device execution in a subprocess.
        _run_in_subprocess(in_maps, out)
    return out


def _run_in_subprocess(in_maps, out):
    import pickle
    import subprocess
    import sys
    import tempfile

    with tempfile.TemporaryDirectory() as td:
        in_path = f"{td}/in.pkl"
        out_path = f"{td}/out.npy"
        with open(in_path, "wb") as f:
            pickle.dump({"in_maps": in_maps}, f,
                        protocol=pickle.HIGHEST_PROTOCOL)
        for attempt in range(3):
            r = subprocess.run(
                [sys.executable, __file__, "--worker", in_path, out_path],
                capture_output=True)
            if r.returncode == 0:
                break
            if attempt == 2:
                raise RuntimeError(
                    f"device worker failed 3x: {r.stderr[-2000:]!r}")
        out[:] = np.load(out_path)


def _worker_main(in_path, out_path):
    import pickle
    with open(in_path, "rb") as f:
        job = pickle.load(f)
    nc = _get_nc()
    res = run_bass_kernel_spmd(nc, job["in_maps"],
                               core_ids=list(range(N_CORES)))
    out = np.empty((B_FULL, DM), dtype=np.float32)
    for i in range(N_CORES):
        out[i * BS:(i + 1) * BS] = np.asarray(
            res.results[i]["out"], dtype=np.float32)
    np.save(out_path, out)


if __name__ == "__main__":
    import sys as _sys
    if len(_sys.argv) == 4 and _sys.argv[1] == "--worker":
        _worker_main(_sys.argv[2], _sys.argv[3])
